# revision 16
# baseline (speedup 1.0000x reference)
"""BMMRemapper Trainium2 kernel (v9, hybrid gather).

Math: out[n,c,q] = sum_k x[n,c,k] * mat[n,q,k]; mat has 4 nonzeros per row q
(bilinear corners lin, lin+1, lin+48, lin+49 with weights (1-a)(1-b), (1-a)b,
a(1-b), ab, zeroed by an all-batch disk mask).

Per core (batch-parallel, N=8 = n_cores, no cross-core comms) the kernel
gathers one 1 KB quad row (4 corner rows x 128 ch, fp16) per output pixel
from a staged quad table, then applies the bilinear weights on DVE.

The gather is descriptor-generation-bound on the Q7 (SWDGE) cores
(~9 ns/pixel), so it is split across two mechanisms that run on DIFFERENT
Q7 core pairs:
  - tiles 0-5: six indirect_dma_start calls (INDIRECT1D, standard DGE
    ucode, core pair 0). No library load needed, so these start as soon as
    the pixel-major indices are ready (~11 us) and feed the DVE early.
  - tiles 6-17: three dma_gather chunks on SWDGE queues 1-3. These
    dispatch in ~70 ns each and generate descriptors asynchronously on
    core pairs 1-3 (3-way parallel), but first use is gated by the mlp
    library IRAM overlay load (~10 us) which runs concurrently with the
    indirect phase.

Combine in fp16 with three wide TT passes per chunk, all in DVE 2x_1p mode
(2 elem/cycle). Per-(p,t) weights stay fast-mode-eligible via duplication:
each weight stored twice adjacently (innermost AP dim [1,2], step 1) with
channel broadcast on stride-0 middle dims:
      m = G * Wdup     [p, (t, rv, rh, c2, j)]
      s = m_rv0 + m_rv1
      o = s_rh0 + s_rh1  -> store
Mask folded into the weights; output stored fp16, host upcasts.

Layouts (q = output pixel, 0..2303; t = q//128; p = q%128):
  xq     (2304, 512) fp16 : quad row k = [x^T[k]|x^T[k+1]|x^T[k+48]|x^T[k+49]].
  gcoef  (128, 36)   f32  : own-batch grid, [p, 2*t+coord].
  gidx   (128, 288)  f32  : grid replicated in dma_gather's 16-partition-
                            wrapped order: [p, 2s+coord], s = t*8 + p'//16,
                            pixel p' = (s%8)*16 + p%16.
  gall   (128, 288)  f32  : all-batch grid, [p, 16*t + 2*m + coord].
  outp   (128, 2304) fp16 : [p, t*128 + c]  (host re-permutes to (c, q)).
"""

import numpy as np

N, H, W, C = 8, 48, 48, 128
HW = H * W            # 2304
NT = HW // 128        # 18
EPS = 1e-5
CLIP_HI = float(np.float32(float(H - 1) - EPS))  # 46.99999 (f32)

# dma_gather chunks (t0, t1, swdge_queue); queues 1-3 dispatch async and
# generate descriptors in parallel on Q7 core pairs 1-3 (3 waves of 3)
GCHUNKS = [(0, 2, 1), (2, 4, 2), (4, 6, 3), (6, 8, 1), (8, 10, 2),
           (10, 12, 3), (12, 14, 1), (14, 16, 2), (16, 18, 3)]

_CACHE = {}


def _build_nc():
    from contextlib import ExitStack

    import concourse.bacc as bacc
    import concourse.bass as bass
    import concourse.mybir as mybir
    import concourse.tile as tile

    dt = mybir.dt
    f32, f16, i16 = dt.float32, dt.float16, dt.int16
    i32 = dt.int32
    Alu = mybir.AluOpType

    nc = bacc.Bacc(
        "TRN2", target_bir_lowering=False, debug=False, num_devices=N,
        num_swdge_queues=4,
    )

    xq = nc.dram_tensor("xq", [HW, 4 * C], f16, kind="ExternalInput")
    gcoef = nc.dram_tensor("gcoef", [128, 2 * NT], f32, kind="ExternalInput")
    gidx = nc.dram_tensor("gidx", [128, 2 * 8 * NT], f32, kind="ExternalInput")
    gall = nc.dram_tensor("gall", [128, 16 * NT], f32, kind="ExternalInput")
    ident = nc.dram_tensor("ident", [128, 128], f16, kind="ExternalInput")
    outp = nc.dram_tensor("outp", [128, HW], f16, kind="ExternalOutput")

    from concourse.library_config import mlp

    with tile.TileContext(nc) as tc, ExitStack() as ctx:
        pool = ctx.enter_context(tc.tile_pool(name="p", bufs=1))
        ppool = ctx.enter_context(tc.tile_pool(name="ps", bufs=1, space="PSUM"))

        # ---- load the dma_gather ucode library FIRST: the IRAM overlay
        # fetch (~10 us) then runs in the background while the input loads,
        # index chains and the indirect gathers proceed ----
        nc.gpsimd.load_library(mlp)

        # ---- input loads (gidx first: it gates the gathers) ----
        g_idx = pool.tile([128, 16 * NT], f32)
        nc.sync.dma_start(g_idx[:], gidx.ap())
        g_coef = pool.tile([128, 2 * NT], f32)
        nc.sync.dma_start(g_coef[:], gcoef.ap())
        g_all = pool.tile([128, 16 * NT], f32)
        nc.sync.dma_start(g_all[:], gall.ap())
        id_sb = pool.tile([128, 128], f16)
        nc.sync.dma_start(id_sb[:], ident.ap())

        # fast floor: int cast of (x - 0.5) rounds-to-nearest on HW, which is
        # exact floor for clipped x (CoreSim truncates -> sim numerics differ,
        # but stay in-bounds; HW is ground truth).

        # ---- wrapped int16 indices for the dma_gather path ----
        cab2 = pool.tile([128, 16 * NT], f32)
        nc.vector.tensor_scalar(cab2[:], g_idx[:], EPS, CLIP_HI, Alu.max, Alu.min)
        ti2 = pool.tile([128, 16 * NT], i32)
        nc.vector.tensor_scalar(ti2[:], cab2[:], -0.5, None, Alu.add)
        idx16 = pool.tile([128, 8 * NT], i16)
        nc.vector.scalar_tensor_tensor(
            idx16[:], ti2[:, 0::2], float(W), ti2[:, 1::2], Alu.mult, Alu.add
        )

        # ---- gathers ----
        gts = []
        for ci, (t0, t1, qn) in enumerate(GCHUNKS):
            k = t1 - t0
            gt_c = pool.tile([128, k * 512], f16, tag=f"G{ci}")
            nc.gpsimd.dma_gather(
                out_ap=gt_c[:].rearrange("p (t e) -> p t e", e=512),
                in_ap=xq.ap(),
                idxs_ap=idx16[:, 8 * t0 : 8 * t1],
                num_idxs=128 * k,
                num_idxs_reg=128 * k,
                elem_size=512,
                queue_num=qn,
            )
            gts.append(gt_c)

        # ---- coefficient chain ([128, NT] per quantity) ----
        cab = pool.tile([128, 2 * NT], f32)
        nc.vector.tensor_scalar(cab[:], g_coef[:], EPS, CLIP_HI, Alu.max, Alu.min)
        tic = pool.tile([128, 2 * NT], i32)
        nc.vector.tensor_scalar(tic[:], cab[:], -0.5, None, Alu.add)
        tf = pool.tile([128, 2 * NT], f32)
        nc.vector.tensor_copy(tf[:], tic[:])

        # mask: AND over all batches of in-bounds test
        g_all3 = g_all[:].rearrange("p (t m) -> p t m", m=16)
        mn = pool.tile([128, NT], f32)
        mx = pool.tile([128, NT], f32)
        nc.vector.tensor_reduce(mn[:], g_all3, mybir.AxisListType.X, Alu.min)
        nc.vector.tensor_reduce(mx[:], g_all3, mybir.AxisListType.X, Alu.max)
        mge = pool.tile([128, NT], f32)
        mle = pool.tile([128, NT], f32)
        nc.vector.tensor_scalar(mge[:], mn[:], -0.5, None, Alu.is_ge)
        nc.vector.tensor_scalar(mle[:], mx[:], float(H) - 0.5, None, Alu.is_le)
        mask = pool.tile([128, NT], f32)
        nc.vector.tensor_tensor(mask[:], mge[:], mle[:], Alu.mult)

        # weights, mask folded in
        fr = pool.tile([128, 2 * NT], f32)   # fractions (a, b interleaved)
        nc.vector.tensor_tensor(fr[:], cab[:], tf[:], Alu.subtract)
        a = fr[:, 0::2]
        b = fr[:, 1::2]
        fb0 = pool.tile([128, NT], f32)  # 1-b
        nc.vector.tensor_scalar(fb0[:], b, -1.0, 1.0, Alu.mult, Alu.add)
        fa0 = pool.tile([128, NT], f32)  # 1-a
        nc.vector.tensor_scalar(fa0[:], a, -1.0, 1.0, Alu.mult, Alu.add)
        am = pool.tile([128, NT], f32)   # a*mask
        a0m = pool.tile([128, NT], f32)  # (1-a)*mask
        nc.vector.tensor_tensor(am[:], a, mask[:], Alu.mult)
        nc.vector.tensor_tensor(a0m[:], fa0[:], mask[:], Alu.mult)

        w4 = []
        for nm, wa, wb in (("w00", a0m, fb0), ("w01", a0m, None),
                           ("w10", am, fb0), ("w11", am, None)):
            wt = pool.tile([128, NT], f32, tag=nm)
            nc.vector.tensor_tensor(
                wt[:], wa[:], wb[:] if wb is not None else b, Alu.mult
            )
            w4.append(wt)

        # wd[p, 8t + 4rv + 2rh + j] = w_{rv,rh}[p, t] (fp16, duplicated j=0,1
        # so the combine's weight AP has innermost [1,2] -> DVE 2x_1p mode)
        wd = pool.tile([128, 8 * NT], f16)
        for r, wt in enumerate(w4):
            nc.vector.tensor_copy(
                wd[:].rearrange("p (t r j) -> p t r j", r=4, j=2)[:, :, r, :],
                wt[:].rearrange("p (t j) -> p t j", j=1).broadcast_to([128, NT, 2]),
            )

        # ---- combine: DVE does the weight-multiply (m) pass; the corner
        # sum runs on the otherwise-idle PE as 4 PSUM-accumulated identity
        # matmuls; the idle ACT engine evacuates PSUM -> fp16 output ----
        o_a = pool.tile([128, 8 * 128], f16, tag="o_a")
        o_b = pool.tile([128, 10 * 128], f16, tag="o_b")

        for ci, (t0, t1, _qn) in enumerate(GCHUNKS):
            k = t1 - t0
            g5 = gts[ci][:].rearrange(
                "p (t rv rh c2 j) -> p t rv rh c2 j", t=k, rv=2, rh=2, c2=64, j=2
            )
            wd5 = (
                wd[:, 8 * t0 : 8 * t1]
                .rearrange("p (t rv rh j) -> p t rv rh j", rv=2, rh=2, j=2)
                .unsqueeze(4)
                .broadcast_to([128, k, 2, 2, 64, 2])
            )
            m = pool.tile([128, k * 512], f16, tag=f"m{ci}")
            m5 = m[:].rearrange(
                "p (t rv rh c2 j) -> p t rv rh c2 j", t=k, rv=2, rh=2, c2=64, j=2
            )
            nc.vector.tensor_tensor(m5, g5, wd5, Alu.mult)

            ps = ppool.tile([128, k * 128], f32, tag=f"ps{ci % 4}")
            m4 = m[:].rearrange("p (t r c) -> p t r c", r=4, c=128)
            for r in range(4):
                nc.tensor.matmul(
                    out=ps[:].rearrange("p (t c) -> p t c", c=128),
                    lhsT=id_sb[:],
                    rhs=m4[:, :, r, :],
                    start=(r == 0),
                    stop=(r == 3),
                )

            ob, b0 = (o_a, 0) if t1 <= 8 else (o_b, 8)
            nc.scalar.activation(
                ob[:, 128 * (t0 - b0) : 128 * (t1 - b0)],
                ps[:],
                mybir.ActivationFunctionType.Copy,
            )
            if t1 == 8:
                nc.sync.dma_start(outp.ap()[:, : 128 * 8], o_a[:])
            elif t1 == NT:
                nc.sync.dma_start(outp.ap()[:, 128 * 8 :], o_b[:])

    nc.compile()
    return nc


def _get_nc():
    if "nc" not in _CACHE:
        _CACHE["nc"] = _build_nc()
    return _CACHE["nc"]


def _stage_inputs(x, grid):
    """Build the per-core input maps (pure data movement / fp16 cast)."""
    x = np.ascontiguousarray(x, dtype=np.float32)
    grid = np.ascontiguousarray(grid, dtype=np.float32)
    xr = x.reshape(N, C, HW)
    gr = grid.reshape(N, HW, 2)

    # quad-row table in fp16: xq[n][k] = [xT[k], xT[k+1], xT[k+48], xT[k+49]]
    xt = np.zeros((N, HW + W + 2, C), dtype=np.float32)
    xt[:, :HW] = xr.transpose(0, 2, 1)
    xq = np.empty((N, HW, 4 * C), dtype=np.float16)
    xq[:, :, 0 * C : 1 * C] = xt[:, 0:HW]
    xq[:, :, 1 * C : 2 * C] = xt[:, 1 : HW + 1]
    xq[:, :, 2 * C : 3 * C] = xt[:, W : HW + W]
    xq[:, :, 3 * C : 4 * C] = xt[:, W + 1 : HW + W + 1]

    # gcoef[n][p, 2t+c] = gr[n, t*128+p, c]
    gc = gr.reshape(N, NT, 128, 2).transpose(0, 2, 1, 3)  # [n, p, t, c]
    gcoef = np.ascontiguousarray(gc.reshape(N, 128, 2 * NT))

    # gidx[p, 2s+c] = gr[q(s, p%16), c], q(s, r) = (s//8)*128 + (s%8)*16 + r
    s_ = np.arange(8 * NT)
    r_ = np.arange(16)
    qm = (s_[None, :] // 8) * 128 + (s_[None, :] % 8) * 16 + r_[:, None]  # [16,144]
    gidx16 = gr[:, qm, :].reshape(N, 16, 2 * 8 * NT)          # [n, 16, 288]
    gidx = np.ascontiguousarray(np.tile(gidx16, (1, 8, 1)))   # [n, 128, 288]

    # gall[p, 16t+2m+c] = gr[m, t*128+p, c]   (same for all cores)
    ga = gr.reshape(N, NT, 128, 2).transpose(2, 1, 0, 3)  # [p, t, m, c]
    gall = np.ascontiguousarray(ga.reshape(128, 16 * NT))

    ident = np.eye(128, dtype=np.float16)
    return [
        {"xq": xq[n], "gcoef": gcoef[n], "gidx": gidx[n], "gall": gall,
         "ident": ident}
        for n in range(N)
    ]


def _unstage_output(results):
    """results[n]["outp"] is (128, 2304) fp16 = [p, t*128+c] -> (N, C, H, W)."""
    out = np.empty((N, C, H, W), dtype=np.float32)
    for n in range(N):
        o = results[n]["outp"].astype(np.float32).reshape(128, NT, C)
        out[n] = o.transpose(2, 1, 0).reshape(C, H, W)   # [c, q=t*128+p]
    return out


def kernel(x, grid):
    from concourse import bass_utils

    nc = _get_nc()
    in_maps = _stage_inputs(x, grid)
    res = bass_utils.run_bass_kernel_spmd(nc, in_maps, core_ids=list(range(N)))
    return _unstage_output(res.results)
